# revision 63
# baseline (speedup 1.0000x reference)
"""Causal self-attention (B=4, T=2048, C=1024, 16 heads) on 8 Trainium2 cores.

Sharding: batch x head-group.  Core c handles batch b = c//2 and head group
hg = c%2 (8 heads = 4 head-pairs).  Each core computes q/k/v projections for
its heads, causal attention, and a partial output projection; the host sums
the two partials per batch at gather.

v7 (vs v4 at 396us HW, measured with the R=8/16 unroll-slope method; v7
measures ~244-255us): microbenchmark-driven restructure.  Measured on HW:
K=64 matmuls run at HALF rate (433ns vs 268ns for K=128 @ F=512), each
matmul costs ~17-55ns issue overhead, fp8 DoubleRow = same wall as fp16
(2x flops, so no win at fixed accuracy -- and fp8 data anywhere in the
attention math fails the 2e-2 gate outright), f32r ~6% slower than fp16.
  * QK at K=128: per strip, one matmul per head with rhs = qz, the q tile
    zero-interleaved per head ([0:64] rows live for h=0, [64:128] for
    h=1, other half zero) so cross-head contraction terms vanish.  2x
    real QK rate vs the natural K=64 form.
  * PV in [t, ch] orientation: per (si, head, 128-t-block) F=65 matmuls
    (64 v-channels + ones column accumulating the softmax denominator).
    Denominators land as PSUM columns; normalization is a [128,4]
    reciprocal + strided tensor_mul with stride-0 broadcast (replaces
    v4's lane-starved [1,512] reciprocals + gpsimd partition_broadcasts).
    PSUM accumulation groups are per-2KB-bank: each yps bank is opened by
    one zero matmul (start=True), PV accumulates with start=False and a
    single bank-last stop (multiple start=True per bank zeroes the whole
    bank on HW -> NaN).
  * y [t,ch] -> yT [ch,t] via DMA XBAR transpose (SBUF->SBUF, 16-bit):
    zero PE/PSUM/DVE cost, runs on idle DMA engines; transpose chains are
    carried into the NEXT pair/attention-phase's slots (emitted before
    any filler that reads yT).  PE is_transpose fallback kept behind
    OPTS["dma_transpose"].
  * Attention emission is software-pipelined: QK(strip i+1) is emitted
    between exp(i) and PV(i), so PV never waits on the exp round-trip.
  * x/w_qkv/w_proj in fp16 (host-converted): halves input DMA.
  * Causal mask: multiplicative fp16 0/1 on the probability tile (DVE),
    off the critical path thanks to the pipelined emission.
  * PSUM: scores 2x2 banks (shared with transpose tiles), y 2x1 banks,
    qkv/proj 2-bank rotation.  Fillers (proj(j-1) interleaved with
    b(j+1), next-rep B(0)) spread across attention slots; next-rep x
    DMAs pre-issued one phase early.
"""
import numpy as np

import concourse.bass as bass
import concourse.tile as tile
from concourse import mybir, bacc
from concourse.bass_utils import run_bass_kernel_spmd

f32 = mybir.dt.float32
f16 = mybir.dt.float16
Exp = mybir.ActivationFunctionType.Exp

B, T, C = 4, 2048, 1024
N_HEAD = 16
D = C // N_HEAD                 # 64
HPC = N_HEAD // 2               # heads per core = 8
NPAIR = HPC // 2                # head pairs per core = 4
CO_Q = C // 2                   # q channels per core = 512
CT = C // 128                   # contraction tiles for qkv = 8
TJ = T // 512                   # t super-tiles = 4
NS = T // 128                   # s tiles = 16
SCALE = float(D) ** -0.5        # 0.125

_CACHE = {}

# memset_open=True (DVE memset as PSUM accumulation base) FAULTS real HW
# (NRT_EXEC_UNIT_UNRECOVERABLE) despite passing CoreSim: keep the PE
# zero-matmul opener.  act_evp untested on device; keep off.
OPTS = {"xrep": True, "pool_mask": False, "dma_transpose": True,
        "memset_open": False, "act_evp": False, "f1_open": True}


def _regn(t, col0, w, n, span=512):
    """n-region AP: columns [col0+k*span : col0+k*span+w] for k<n."""
    base = t[:, col0:col0 + 1]
    return bass.AP(tensor=base.tensor, offset=base.offset,
                   ap=[t.ap[0], [span, n], [1, w]])


class _RepState:
    def __init__(self, rep):
        self.rep = rep
        self.xr = {}
        self.yT = {}
        self.o = {}


def _build_nc(reps=1):
    import contextlib
    from collections import deque

    nc = bacc.Bacc("TRN2", target_bir_lowering=False, debug=False)
    xT_d = nc.dram_tensor("xT", [C, T], f16, kind="ExternalInput").ap()
    wqT_d = nc.dram_tensor("wqT", [C, CO_Q], f16, kind="ExternalInput").ap()
    wkT_d = nc.dram_tensor("wkT", [C, CO_Q], f16, kind="ExternalInput").ap()
    wvT_d = nc.dram_tensor("wvT", [C, CO_Q], f16, kind="ExternalInput").ap()
    wpT_d = nc.dram_tensor("wpT", [CO_Q, C], f16, kind="ExternalInput").ap()
    bmask_d = nc.dram_tensor("bmask", [128, 128], f16, kind="ExternalInput").ap()
    ident_d = nc.dram_tensor("ident", [128, 128], f16, kind="ExternalInput").ap()
    out_d = nc.dram_tensor("out", [T, C], f32, kind="ExternalOutput").ap()

    def dma(out, in_):
        nc.sync.dma_start(out=out, in_=in_)

    with tile.TileContext(nc) as tc, contextlib.ExitStack() as ctx:
        ep = ctx.enter_context
        persist = ep(tc.tile_pool(name="persist", bufs=1))
        xin = ep(tc.tile_pool(name="xin", bufs=2))
        pw = ep(tc.tile_pool(name="pw", bufs=2))
        ypool = ep(tc.tile_pool(name="ypool", bufs=2))
        rp = ep(tc.tile_pool(name="rp", bufs=2))
        ytp = ep(tc.tile_pool(name="ytp", bufs=2))
        ob = ep(tc.tile_pool(name="ob", bufs=2))
        qvp = ep(tc.tile_pool(name="qvp", bufs=2, space="PSUM"))
        sps = ep(tc.tile_pool(name="sps", bufs=2, space="PSUM"))
        ypsp = ep(tc.tile_pool(name="ypsp", bufs=1, space="PSUM"))

        # ---- persistent tensors, shared by all reps (WAR-chained) ----
        # qz[:, p, j, h, :]: rows 0:64 of h=0 hold qA, rows 64:128 of h=1
        # hold qB, other halves stay zero -> one K=128 matmul per strip
        # computes both heads' scores.
        qz = persist.tile([128, NPAIR, TJ, 2, 512], f16)
        kT = persist.tile([128, NPAIR, TJ, 512], f16)
        # vAB[:, p, si, :] = [VA(64) | 1 | VB(64) | 1]
        vAB = persist.tile([128, NPAIR, NS, 130], f16)
        bmask = persist.tile([128, 128], f16)
        ident = persist.tile([128, 128], f16)
        wq_sb = persist.tile([128, CT, CO_Q], f16)
        wk_sb = persist.tile([128, CT, CO_Q], f16)
        wv_sb = persist.tile([128, CT, CO_Q], f16)
        wpT_r = persist.tile([128, NPAIR, C], f16)
        z128 = persist.tile([128, 260], f16)

        # ---- one-time init (mask, identity, ones columns, qz zeros) ----
        dma(out=bmask[:], in_=bmask_d[:, :])
        dma(out=ident[:], in_=ident_d[:, :])
        nc.vector.memset(z128[:], 0.0)
        nc.vector.memset(qz[:], 0.0)
        for p in range(NPAIR):
            nc.vector.memset(vAB[:, p, :, 64:65], 1.0)
            nc.vector.memset(vAB[:, p, :, 129:130], 1.0)

        def emit_x_dma(s, j):
            xr = xin.tile([128, CT, 512], f16, tag="xr", name=f"xr{s.rep}{j}")
            src = bass.AP(tensor=xT_d.tensor, offset=j * 512,
                          ap=[[T, 128], [128 * T, CT], [1, 512]])
            dma(out=xr[:], in_=src)
            s.xr[j] = xr

        def wdma_pieces(s):
            def wdmas():
                for wsb, wd in ((wq_sb, wqT_d), (wk_sb, wkT_d), (wv_sb, wvT_d)):
                    src = bass.AP(tensor=wd.tensor, offset=0,
                                  ap=[[CO_Q, 128], [128 * CO_Q, CT], [1, CO_Q]])
                    dma(out=wsb[:], in_=src)
            return [wdmas]

        def setup_dma_pieces(s):
            return [lambda: emit_x_dma(s, 0)] + wdma_pieces(s)

        def wp_stage_pieces(s):
            pieces = []
            for p in range(NPAIR):
                def f(p=p):
                    dma(out=wpT_r[:, p, :], in_=wpT_d[p * 128:(p + 1) * 128, :])
                pieces.append(f)
            return pieces

        def b_group_pieces(s, j, kind, p_or_sj):
            """Closures for one qkv/V group: 4 matmul chunks + evict."""
            pieces = []
            if kind in ("q", "k"):
                i, p = p_or_sj
                holder = {}

                def mk(ct0):
                    def f():
                        if ct0 == 0:
                            holder["ps"] = qvp.tile([128, 512], f32, tag="qv",
                                                    name=f"qk{s.rep}{j}{p}")
                        ps = holder["ps"]
                        wsb = (wq_sb, wk_sb)[i]
                        for ct in (ct0, ct0 + 1):
                            nc.tensor.matmul(
                                ps[:], wsb[:, ct, p * 128:(p + 1) * 128],
                                s.xr[j][:, ct, :],
                                start=(ct == 0), stop=(ct == CT - 1))
                    return f
                for ct0 in range(0, CT, 2):
                    pieces.append(mk(ct0))

                if i == 0:
                    def ev():
                        ps = holder["ps"]
                        nc.vector.tensor_copy(qz[0:64, p, j, 0, :], ps[0:64, :])
                        nc.scalar.copy(qz[64:128, p, j, 1, :], ps[64:128, :])
                else:
                    def ev():
                        nc.vector.tensor_copy(kT[:, p, j, :], holder["ps"][:])
                pieces.append(ev)
            else:  # V group: out [t-block, 512 vch]
                sj = p_or_sj
                holder = {}

                def mkv(ct0):
                    def f():
                        if ct0 == 0:
                            holder["ps"] = qvp.tile([128, 512], f32, tag="qv",
                                                    name=f"v{s.rep}{j}{sj}")
                        ps = holder["ps"]
                        for ct in (ct0, ct0 + 1):
                            nc.tensor.matmul(
                                ps[:], s.xr[j][:, ct, sj * 128:(sj + 1) * 128],
                                wv_sb[:, ct, :],
                                start=(ct == 0), stop=(ct == CT - 1))
                    return f
                for ct0 in range(0, CT, 2):
                    pieces.append(mkv(ct0))

                def evv():
                    si = j * 4 + sj
                    ps = holder["ps"]
                    for p in range(NPAIR):
                        sb = ps[:, p * 128:p * 128 + 1]
                        src = bass.AP(tensor=sb.tensor, offset=sb.offset,
                                      ap=[ps.ap[0], [64, 2], [1, 64]])
                        db = vAB[:, p, si, 0:1]
                        dst = bass.AP(tensor=db.tensor, offset=db.offset,
                                      ap=[vAB.ap[0], [65, 2], [1, 64]])
                        nc.vector.tensor_copy(dst, src)
                pieces.append(evv)
            return pieces

        def phase_b_pieces(s, j):
            def xdma():
                if j not in s.xr:
                    emit_x_dma(s, j)
            pieces = [xdma]
            for p in range(NPAIR):
                pieces += b_group_pieces(s, j, "q", (0, p))
                pieces += b_group_pieces(s, j, "k", (1, p))
            for sj in range(4):
                pieces += b_group_pieces(s, j, "v", sj)
            return pieces

        def b0_q_pieces(s):
            pieces = []
            for p in range(NPAIR):
                pieces += b_group_pieces(s, 0, "q", (0, p))
            return pieces

        def b0_k_pieces_pp(s):
            """Next-rep k(0) groups, one list per pair: pair p's group only
            overwrites kT[:, p, 0, :], whose attention(3) readers are that
            pair's strips si 0..3 — safe to emit from (p, si>=4)."""
            return [b_group_pieces(s, 0, "k", (1, p)) for p in range(NPAIR)]

        def b0_v_pieces(s):
            pieces = []
            for sj in range(4):
                pieces += b_group_pieces(s, 0, "v", sj)
            return pieces

        def proj_pieces(s, j):
            pieces = []
            for tj in range(4):
                for nh in range(2):
                    holder = {}

                    def mkp(p0, tj=tj, nh=nh, holder=holder):
                        def f():
                            if p0 == 0:
                                holder["ps"] = qvp.tile(
                                    [128, 512], f32, tag="qv",
                                    name=f"pr{s.rep}{j}{tj}{nh}")
                                if nh == 0:
                                    s.o[(j, tj)] = ob.tile(
                                        [128, C], f32, tag="o",
                                        name=f"o{s.rep}{j}{tj}")
                            ps = holder["ps"]
                            yT = s.yT[j]
                            for p in (p0, p0 + 1):
                                nc.tensor.matmul(
                                    ps[:], yT[:, p, tj * 128:(tj + 1) * 128],
                                    wpT_r[:, p, nh * 512:(nh + 1) * 512],
                                    start=(p == 0), stop=(p == NPAIR - 1))
                        return f
                    pieces.append(mkp(0))
                    pieces.append(mkp(2))

                    def evp(tj=tj, nh=nh, holder=holder):
                        o_sb = s.o[(j, tj)]
                        # alternate DVE/ACT so qv-ring clears don't queue
                        # behind each other on one engine
                        eng = (nc.scalar.copy if (OPTS["act_evp"] and nh == 1)
                               else nc.vector.tensor_copy)
                        eng(o_sb[:, nh * 512:(nh + 1) * 512], holder["ps"][:])
                        if nh == 1:
                            row = j * 512 + tj * 128
                            dma(out=out_d[row:row + 128, :], in_=o_sb[:])
                    pieces.append(evp)
            return pieces

        def attention(s, j, fillers, tails, carry, flush_carry=False,
                      tails_pp=None):
            nsj = 4 * (j + 1)
            seq = [(p, si) for p in range(NPAIR) for si in range(nsj)]
            nslot = 2 * len(seq) + 2 * NPAIR
            ntail_slots = nsj - 3
            fi = 0
            ti = 0
            tails_pp = tails_pp or [[] for _ in range(NPAIR)]
            tpi = [0] * NPAIR

            def pop(lst, idx, slots_left):
                want = len(lst) - idx
                if want <= 0:
                    return idx
                n = -(-want // max(slots_left, 1)) if slots_left > 0 else want
                for _ in range(n):
                    if idx < len(lst):
                        lst[idx]()
                        idx += 1
                return idx

            slot = [nslot]
            tslot = [ntail_slots]
            yT = ytp.tile([128, NPAIR, 512], f16, tag="yT", name=f"yT{s.rep}{j}")
            s.yT[j] = yT
            sts, pds, ypss = {}, {}, {}

            def qk(idx):
                p, si = seq[idx]
                if si == 0:
                    ypss[p] = [
                        ypsp.tile([128, 2, 130], f32, tag=f"yps{g}",
                                  name=f"yps{g}_{s.rep}{j}{p}")
                        for g in range(2)]
                    # one accumulation group per PSUM bank: zero each yps
                    # bank off the PE (DVE memset); PV accumulates with
                    # start=False and a single bank-last stop.
                    for g in range(2):
                        if OPTS["memset_open"]:
                            nc.vector.memset(ypss[p][g][:, :, :], 0.0)
                        elif OPTS["f1_open"]:
                            # start=True zeroes the whole 2KB bank on HW,
                            # so a 1-column opener suffices
                            nc.tensor.matmul(ypss[p][g][:, 0, 0:1],
                                             z128[:, 0:128], z128[:, 0:1],
                                             start=True, stop=False,
                                             skip_group_check=True)
                        else:
                            nc.tensor.matmul(ypss[p][g][:, :, :],
                                             z128[:, 0:128], z128[:, 0:260],
                                             start=True, stop=False,
                                             skip_group_check=True)
                lo = max(si * 128 - j * 512, 0)
                st = sps.tile([128, 1024], f32, tag="st")
                sts[idx] = st
                ko, ks = si // 4, (si % 4) * 128
                # K=128 per head (zero-interleaved qz kills the cross-head
                # terms); K=64 matmuls run at half rate on HW.
                for h in range(2):
                    nc.tensor.matmul(
                        st[:, h * 512 + lo:h * 512 + 512],
                        kT[:, p, ko, ks:ks + 128],
                        qz[:, p, j, h, lo:512], start=True, stop=True)

            def expi(idx):
                p, si = seq[idx]
                lo = max(si * 128 - j * 512, 0)
                w = 512 - lo
                st = sts.pop(idx)
                pd = pw.tile([128, 1024], f16, tag="pd")
                pds[idx] = pd
                nc.scalar.activation(_regn(pd, lo, w, 2), _regn(st, lo, w, 2),
                                     Exp, scale=SCALE)
                if si >= 4 * j:
                    mreg = _regn(pd, lo, 128, 2)
                    msrc = bass.AP(tensor=bmask.tensor, offset=bmask.offset,
                                   ap=[bmask.ap[0], [0, 2], [1, 128]])
                    (nc.gpsimd if OPTS["pool_mask"] else
                     nc.vector).tensor_mul(mreg, mreg, msrc)

            def pvi(idx):
                p, si = seq[idx]
                pd = pds.pop(idx)
                yps = ypss[p]
                lo = max(si * 128 - j * 512, 0)
                for h in range(2):
                    for tb in range(lo // 128, 4):
                        nc.tensor.matmul(
                            yps[tb // 2][:, tb % 2, h * 65:(h + 1) * 65],
                            pd[:, h * 512 + tb * 128:h * 512 + (tb + 1) * 128],
                            vAB[:, p, si, h * 65:(h + 1) * 65],
                            start=False,
                            stop=(h == 1 and tb % 2 == 1 and si == 4 * j + tb),
                            skip_group_check=True)

            def finish_pair(p):
                yps = ypss.pop(p)
                rden = rp.tile([128, 2, 2, 2], f32, tag="rden",
                               name=f"rd{s.rep}{j}{p}")
                for g in range(2):
                    src = bass.AP(tensor=yps[g].tensor,
                                  offset=yps[g][:, 0, 64:65].offset,
                                  ap=[yps[g].ap[0], [130, 2], [65, 2]])
                    nc.vector.reciprocal(rden[:, g, :, :], src)
                Y = ypool.tile([128, 4, 128], f16, tag="Y",
                               name=f"Y{s.rep}{j}{p}")
                for g in range(2):
                    o_ap = bass.AP(tensor=Y.tensor,
                                   offset=Y[:, 2 * g, 0:1].offset,
                                   ap=[Y.ap[0], [128, 2], [64, 2], [1, 64]])
                    i_ap = bass.AP(tensor=yps[g].tensor,
                                   offset=yps[g].offset,
                                   ap=[yps[g].ap[0], [130, 2], [65, 2], [1, 64]])
                    r_ap = bass.AP(tensor=rden.tensor,
                                   offset=rden[:, g, 0, 0:1].offset,
                                   ap=[rden.ap[0], [2, 2], [1, 2], [0, 64]])
                    nc.vector.tensor_mul(o_ap, i_ap, r_ap)

                def transp(p=p, Y=Y):
                    if OPTS["dma_transpose"]:
                        # XBAR transpose straight to SBUF: no PE, no PSUM,
                        # no DVE eviction
                        for tb in range(4):
                            nc.sync.dma_start(
                                out=yT[:, p, tb * 128:(tb + 1) * 128],
                                in_=Y[:, tb, :], transpose=True)
                        return
                    # one PSUM tile per 128-block: each transpose needs its
                    # own accumulation-group bank (start zeroes whole banks)
                    for tb in range(4):
                        tt = sps.tile([128, 128], f16, tag="st",
                                      name=f"tt{s.rep}{j}{p}{tb}")
                        nc.tensor.transpose(tt[:], Y[:, tb, :], ident[:])
                        nc.vector.tensor_copy(
                            yT[:, p, tb * 128:(tb + 1) * 128], tt[:])
                carry.append(transp)

            qk(0)
            for idx, (p, si) in enumerate(seq):
                expi(idx)
                if idx + 1 < len(seq):
                    qk(idx + 1)
                # carried transposes first: fillers (e.g. proj of the carried
                # pair's j) may read the yT regions they produce
                if carry:
                    carry.popleft()()
                slot[0] -= 1
                fi = pop(fillers, fi, slot[0])
                pvi(idx)
                if si == nsj - 1:
                    finish_pair(p)
                slot[0] -= 1
                fi = pop(fillers, fi, slot[0])
                if si >= 4:
                    tpi[p] = pop(tails_pp[p], tpi[p], nsj - si)
                if p == NPAIR - 1 and si >= 3:
                    tslot[0] -= 1
                    ti = pop(tails, ti, tslot[0])
            while fi < len(fillers):
                fillers[fi]()
                fi += 1
            for p in range(NPAIR):
                while tpi[p] < len(tails_pp[p]):
                    tails_pp[p][tpi[p]]()
                    tpi[p] += 1
            while ti < len(tails):
                tails[ti]()
                ti += 1
            if flush_carry:
                while carry:
                    carry.popleft()()

        def interleave(a, b):
            out = []
            for i in range(max(len(a), len(b))):
                if i < len(a):
                    out.append(a[i])
                if i < len(b):
                    out.append(b[i])
            return out

        # ---- schedule ----
        states = [_RepState(r) for r in range(reps)]
        carry = deque()
        for r in range(reps):
            s = states[r]
            if r == 0:
                for piece in setup_dma_pieces(s) + wp_stage_pieces(s):
                    piece()
                for piece in b0_q_pieces(s):
                    piece()
                for pp in b0_k_pieces_pp(s):
                    for piece in pp:
                        piece()
                for piece in b0_v_pieces(s):
                    piece()
            else:
                for piece in wp_stage_pieces(s):
                    piece()
            for j in range(TJ):
                fillers, tails = [], []
                tails_pp = None
                # proj pieces first (no DMA dependency), b(j+1) interleaved
                pj = []
                if j - 1 >= 0:
                    pj = proj_pieces(s, j - 1)
                elif r > 0:
                    pj = proj_pieces(states[r - 1], TJ - 1)
                bj = phase_b_pieces(s, j + 1) if j + 1 < TJ else []
                if bj:
                    fillers += [bj[0]]  # x DMA first
                    bj = bj[1:]
                fillers += interleave(pj, bj)
                if j == TJ - 2 and r + 1 < reps and OPTS["xrep"]:
                    # pre-issue next rep's x(0) DMA: its xin slot (shared
                    # with x(j=2)) is free once b(2) groups finished
                    fillers += [lambda nxt=states[r + 1]: emit_x_dma(nxt, 0)]
                if j == TJ - 1 and r + 1 < reps and OPTS["xrep"]:
                    nxt = states[r + 1]
                    fillers += wdma_pieces(nxt) + b0_q_pieces(nxt)
                    # pre-issue next rep's x(1) too (slot shared with x(3),
                    # whose readers finished during attention(2))
                    fillers += [lambda nxt=nxt: emit_x_dma(nxt, 1)]
                    for pp in b0_k_pieces_pp(nxt):
                        tails += pp
                    tails += b0_v_pieces(nxt)
                attention(s, j, fillers, tails, carry,
                          flush_carry=(j == TJ - 1 and r == reps - 1),
                          tails_pp=tails_pp)
            if r + 1 < reps and not OPTS["xrep"]:
                nxt = states[r + 1]
                for piece in setup_dma_pieces(nxt) + b0_q_pieces(nxt):
                    piece()
                for pp in b0_k_pieces_pp(nxt):
                    for piece in pp:
                        piece()
                for piece in b0_v_pieces(nxt):
                    piece()
            if r == reps - 1:
                for piece in proj_pieces(s, TJ - 1):
                    piece()

    nc.compile()
    return nc


def _get_nc(reps=1):
    key = f"nc{reps}"
    if key not in _CACHE:
        _CACHE[key] = _build_nc(reps)
    return _CACHE[key]


def make_in_maps(x, w_qkv, w_proj):
    """Shard full inputs into the 8 per-core input maps."""
    x = np.asarray(x, dtype=np.float32)
    w_qkv = np.asarray(w_qkv, dtype=np.float32)
    w_proj = np.asarray(w_proj, dtype=np.float32)
    row = np.arange(128)[:, None]
    col = np.arange(128)[None, :]
    bmask = (row <= col).astype(np.float16)
    ident = (row == col).astype(np.float16)
    in_maps = []
    for c in range(8):
        b, hg = c // 2, c % 2
        sl = slice(hg * CO_Q, (hg + 1) * CO_Q)
        in_maps.append({
            "xT": np.ascontiguousarray(x[b].T).astype(np.float16),
            "wqT": np.ascontiguousarray(w_qkv[0 * C:1 * C][sl].T).astype(np.float16),
            "wkT": np.ascontiguousarray(w_qkv[1 * C:2 * C][sl].T).astype(np.float16),
            "wvT": np.ascontiguousarray(w_qkv[2 * C:3 * C][sl].T).astype(np.float16),
            "wpT": np.ascontiguousarray(w_proj[:, sl].T).astype(np.float16),
            "bmask": bmask,
            "ident": ident,
        })
    return in_maps


def gather(results):
    """Sum the two head-group partials per batch, stack batches."""
    out = np.empty((B, T, C), dtype=np.float32)
    for b in range(B):
        out[b] = results[2 * b]["out"] + results[2 * b + 1]["out"]
    return out


def kernel(x, w_qkv, w_proj):
    nc = _get_nc()
    in_maps = make_in_maps(x, w_qkv, w_proj)
    res = run_bass_kernel_spmd(nc, in_maps, core_ids=list(range(8)))
    return gather(res.results)
